# revision 15
# baseline (speedup 1.0000x reference)
"""Fused sparse-attention CNN kernel for TRN2 (8 NeuronCores, batch-parallel).

Per batch b (one per core), with L=2048, H=128:
  cos[l,m] = <s_l, s_m> / (|s_l||s_m|)  masked to band (m <= l+2, diag removed
  except (0,0)); att = softmax over l (per-column normalization);
  x2 = att @ x; GLU over concat([x, x2, individual]); 3x causal conv1d(K=3)
  + relu; times next_skill.

Key structure exploited on-chip:
  - softmax normalizes over full columns m, so att = E / colsum(E) with
    E = exp(masked cos) and x2 = E^T-layout matmul with x pre-scaled by
    1/colsum. No online softmax needed.
  - E is stored transposed, ET[m, l], so colsums are free-axis reductions
    (fused into the exp pass via accum_out) and both big matmuls contract
    over the partition axis naturally.
  - the band mask means tiles with l < m-2 are skipped entirely (~1/3 of
    the [L, L] work) and only diagonal-adjacent tiles need mask multiplies.
  - everything after the cos matmul stays in SBUF; the [L, L] tensor never
    touches HBM.
"""

import numpy as np

import concourse.bacc as bacc
import concourse.bass as bass
import concourse.tile as tile
from concourse import mybir
from concourse.bass_utils import run_bass_kernel_spmd
from concourse.masks import make_identity

L = 2048
H = 128
NT = 16  # l/m tiles of 128
B = 8
F32 = mybir.dt.float32
F32R = mybir.dt.float32r
AF = mybir.ActivationFunctionType
ALU = mybir.AluOpType
AX = mybir.AxisListType

# last m-tile index contributing to each 512-wide l-chunk of x2
_LAST_I = [4, 8, 12, 15]


def _c0(i):
    # first 512-chunk of l covered by m-tile i's ET strip
    return min(3, max(0, (128 * i - 2) // 512))


def _p1_chunks(i):
    """Phase-1 cos-matmul chunks for m-tile i: list of (lo, n, mask_kind)."""
    mi = 128 * i
    if i == 0:
        ch = [(0, 512, "mask0")]
        lo = 512
    else:
        ch = [(mi - 128, 256, "maskcd")]
        lo = mi + 128
    rem = L - lo
    while rem > 0:
        n = 384 if rem == 640 else (512 if rem >= 512 else rem)
        ch.append((lo, n, None))
        lo += n
        rem -= n
    return ch


def _strip_layout():
    base = [512 * _c0(i) for i in range(NT)]
    width = [L - b for b in base]
    off = np.concatenate([[0], np.cumsum(width)]).astype(int)
    return base, width, off


def build_nc():
    nc = bacc.Bacc("TRN2", target_bir_lowering=False, debug=False, num_devices=B)

    inp = {}
    for name, shape in [
        ("skills_pt", [H, L]),  # [p, t*128] tile-partitioned natural layout
        ("x_pt", [H, L]),
        ("xT", [H, L]),
        ("indT", [H, L]),
        ("nsT", [H, L]),
        ("mask0", [H, 512]),
        ("maskcd", [H, 256]),
        ("w1_pt", [H, 384]),
        ("w2_pt", [H, 384]),
        ("cw_pt", [H, 384]),
        ("b1", [H, 1]),
        ("b2", [H, 1]),
        ("cb", [H, 1]),
    ]:
        inp[name] = nc.declare_dram_parameter(name, shape, F32, isOutput=False)
    outT = nc.declare_dram_parameter("outT", [H, L], F32, isOutput=True)

    with tile.TileContext(nc) as tc:
        _body(nc, tc, inp, outT)
    nc.compile()
    return nc


def _body(nc, tc, inp, outT):
    base_l, width, off = _strip_layout()
    r32 = lambda ap: ap.bitcast(F32R)  # noqa: E731 (for F32->F32R views)

    with (
        tc.tile_pool(name="persist", bufs=1) as P,
        tc.tile_pool(name="small", bufs=2) as SM,
    ):
        # ---- persistent SBUF buffers ----
        # Anything consumed by an FP32r matmul must be *written* as float32r
        # (walrus BIR verifier requirement), so those tiles are F32R-typed.
        sk = P.tile([H, L], F32, name="sk")  # skills, then sn (in place)
        xn = P.tile([H, L], F32R, name="xn")  # x natural, then xs (in place)
        snT = P.tile([H, L], F32R, name="snT")
        strip = P.tile([H, int(off[NT])], F32R, name="strip")  # ET storage
        xTs = P.tile([H, L], F32R, name="xTs")
        indTs = P.tile([H, L], F32R, name="indTs")
        nsTs = P.tile([H, L], F32, name="nsTs")
        x2T = P.tile([H, L], F32R, name="x2T")
        h0 = P.tile([H, L + 2], F32R, name="h0")  # conv ping
        h1 = P.tile([H, L + 2], F32R, name="h1")  # conv pong
        h2 = P.tile([H, L], F32, name="h2")  # conv3 out (full fp32, no pad)
        m0 = P.tile([H, 512], F32, name="m0")
        mcd = P.tile([H, 256], F32, name="mcd")
        w1t = P.tile([H, 3, H], F32R, name="w1t")
        w2t = P.tile([H, 3, H], F32R, name="w2t")
        cwt = P.tile([H, 3, H], F32R, name="cwt")
        b1t = P.tile([H, 1], F32, name="b1t")
        b2t = P.tile([H, 1], F32, name="b2t")
        cbt = P.tile([H, 1], F32, name="cbt")
        ident = P.tile([H, H], F32, name="ident")
        norm2 = P.tile([H, NT], F32, name="norm2")
        inv_n = P.tile([H, NT], F32, name="inv_n")
        ssub = P.tile([H, NT, 8], F32, name="ssub")
        s_inv = P.tile([H, NT], F32, name="s_inv")
        sqs = P.tile([H, H], F32, name="sqs")

        # ---- input DMAs ----
        nc.sync.dma_start(out=sk, in_=inp["skills_pt"][:, :])
        nc.sync.dma_start(out=xn, in_=inp["x_pt"][:, :].bitcast(F32R))
        nc.sync.dma_start(out=xTs, in_=inp["xT"][:, :].bitcast(F32R))
        nc.sync.dma_start(out=indTs, in_=inp["indT"][:, :].bitcast(F32R))
        nc.sync.dma_start(out=nsTs, in_=inp["nsT"][:, :])
        nc.sync.dma_start(out=m0, in_=inp["mask0"][:, :])
        nc.sync.dma_start(out=mcd, in_=inp["maskcd"][:, :])
        nc.sync.dma_start(out=w1t, in_=inp["w1_pt"][:, :].rearrange("p (r h) -> p r h", r=3).bitcast(F32R))
        nc.sync.dma_start(out=w2t, in_=inp["w2_pt"][:, :].rearrange("p (r h) -> p r h", r=3).bitcast(F32R))
        nc.sync.dma_start(out=cwt, in_=inp["cw_pt"][:, :].rearrange("p (k h) -> p k h", k=3).bitcast(F32R))
        nc.sync.dma_start(out=b1t, in_=inp["b1"][:, :])
        nc.sync.dma_start(out=b2t, in_=inp["b2"][:, :])
        nc.sync.dma_start(out=cbt, in_=inp["cb"][:, :])

        make_identity(nc, ident)

        # Zero-fill ET strip regions never written by phase 1, and conv pads.
        # Memset can't encode dtype float32r, so stage zeros in an F32 tile
        # and copy (the copy converts and satisfies the FP32r-writer rule).
        zeros = P.tile([H, 512], F32, name="zeros")
        nc.vector.memset(zeros, 0.0)
        for i in range(1, NT):
            zf = (128 * i - 128) - base_l[i]
            if zf > 0:
                nc.vector.tensor_copy(
                    out=strip[:, int(off[i]) : int(off[i]) + zf], in_=zeros[:, :zf]
                )
        nc.vector.tensor_copy(out=h0[:, 0:2], in_=zeros[:, 0:2])
        nc.vector.tensor_copy(out=h1[:, 0:2], in_=zeros[:, 0:2])

        with tc.tile_pool(name="ps_work", bufs=3, space="PSUM") as PSW:
            # ---- prologue: norms, sn, snT ----
            for t in range(NT):
                nc.scalar.activation(
                    out=sqs,
                    in_=sk[:, 128 * t : 128 * (t + 1)],
                    func=AF.Square,
                    accum_out=norm2[:, t : t + 1],
                )
            nc.scalar.activation(out=norm2, in_=norm2, func=AF.Sqrt)
            nc.vector.reciprocal(out=inv_n, in_=norm2)
            for t in range(NT):
                nc.vector.tensor_scalar_mul(
                    out=sk[:, 128 * t : 128 * (t + 1)],
                    in0=sk[:, 128 * t : 128 * (t + 1)],
                    scalar1=inv_n[:, t : t + 1],
                )
            for t in range(NT):
                tps = PSW.tile([H, H], F32, tag="cos", name=f"tp{t}")
                nc.tensor.transpose(tps, sk[:, 128 * t : 128 * (t + 1)], ident)
                nc.vector.tensor_copy(out=snT[:, 128 * t : 128 * (t + 1)], in_=tps)

            # ---- phase 1: ET strips + colsums ----
            for i in range(NT):
                mi = 128 * i
                lhs = snT[:, mi : mi + 128]
                chunks = _p1_chunks(i)
                for k, (lo, n, mk) in enumerate(chunks):
                    ps = PSW.tile([H, 512], F32, tag="cos")
                    dst = strip[:, int(off[i]) + lo - base_l[i] : int(off[i]) + lo - base_l[i] + n]
                    nc.tensor.matmul(
                        ps[:, :n], lhsT=lhs, rhs=snT[:, lo : lo + n],
                        start=True, stop=True,
                    )
                    acc = ssub[:, i, k : k + 1]
                    if mk is None:
                        nc.scalar.activation(out=dst, in_=ps[:, :n], func=AF.Exp, accum_out=acc)
                    else:
                        nc.scalar.activation(out=ps[:, :n], in_=ps[:, :n], func=AF.Exp)
                        mt = m0[:, :n] if mk == "mask0" else mcd[:, :n]
                        nc.vector.scalar_tensor_tensor(
                            out=dst, in0=ps[:, :n], scalar=1.0, in1=mt,
                            op0=ALU.mult, op1=ALU.mult, accum_out=acc,
                        )
                nch = len(chunks)
                nc.vector.reduce_sum(
                    out=s_inv[:, i : i + 1], in_=ssub[:, i, 0:nch], axis=AX.X
                )
                nc.vector.reciprocal(out=s_inv[:, i : i + 1], in_=s_inv[:, i : i + 1])
                # xs_i = x_i / S_i (in place)
                nc.vector.tensor_scalar_mul(
                    out=xn[:, mi : mi + 128],
                    in0=xn[:, mi : mi + 128],
                    scalar1=s_inv[:, i : i + 1],
                )

            # ---- phase 2: x2T = sum_i xs_i^T-contract ET_i ----
            with tc.tile_pool(name="ps_x2", bufs=1, space="PSUM") as PSX:
                x2ps = [PSX.tile([H, 512], F32, name=f"x2ps{c}") for c in range(4)]
                for i in range(NT):
                    for c in range(_c0(i), 4):
                        rhs = strip[
                            :, int(off[i]) + 512 * c - base_l[i] : int(off[i]) + 512 * c - base_l[i] + 512
                        ]
                        nc.tensor.matmul(
                            x2ps[c],
                            lhsT=xn[:, 128 * i : 128 * (i + 1)],
                            rhs=rhs,
                            start=(i == 0),
                            stop=(i == _LAST_I[c]),
                        )
                for c in range(4):
                    nc.vector.tensor_copy(out=x2T[:, 512 * c : 512 * (c + 1)], in_=x2ps[c])

        # ---- GLU ----
        with tc.tile_pool(name="ps_glu", bufs=1, space="PSUM") as PSM:
            gps = [PSM.tile([H, 512], F32, name=f"gps{c}") for c in range(4)]
            hps = [PSM.tile([H, 512], F32, name=f"hps{c}") for c in range(4)]
            srcs = [xTs, x2T, indTs]
            for r in range(3):
                for c in range(4):
                    nc.tensor.matmul(
                        gps[c], lhsT=w1t[:, r, :],
                        rhs=srcs[r][:, 512 * c : 512 * (c + 1)],
                        start=(r == 0), stop=(r == 2),
                    )
                for c in range(4):
                    nc.tensor.matmul(
                        hps[c], lhsT=w2t[:, r, :],
                        rhs=srcs[r][:, 512 * c : 512 * (c + 1)],
                        start=(r == 0), stop=(r == 2),
                    )
            for c in range(4):
                gate = SM.tile([H, 512], F32, tag="gate")
                nc.scalar.activation(out=gate, in_=gps[c], func=AF.Sigmoid, bias=b1t)
                # h0 = (hps + b2) * gate
                nc.vector.scalar_tensor_tensor(
                    out=h0[:, 2 + 512 * c : 2 + 512 * (c + 1)],
                    in0=hps[c], scalar=b2t, in1=gate,
                    op0=ALU.add, op1=ALU.mult,
                )

        # ---- convs ----
        with tc.tile_pool(name="ps_cv", bufs=4, space="PSUM") as PSC:
            bufs = [h0, h1, h0]
            for layer in range(3):
                src = bufs[layer]
                for c in range(4):
                    cps = PSC.tile([H, 512], F32, tag="cv")
                    for k in range(3):
                        nc.tensor.matmul(
                            cps, lhsT=cwt[:, k, :],
                            rhs=src[:, 512 * c + k : 512 * c + k + 512],
                            start=(k == 0), stop=(k == 2),
                        )
                    if layer < 2:
                        dsl = bufs[layer + 1][:, 2 + 512 * c : 2 + 512 * (c + 1)]
                    else:
                        # last conv feeds only elementwise ops: full fp32 buffer
                        dsl = h2[:, 512 * c : 512 * (c + 1)]
                    nc.scalar.activation(out=dsl, in_=cps, func=AF.Relu, bias=cbt)

            # final: out = h * next_skill
            nc.vector.tensor_mul(out=h2, in0=h2, in1=nsTs)
            nc.sync.dma_start(out=outT[:, :], in_=h2)


_NC = None


def _get_nc():
    global _NC
    if _NC is None:
        _NC = build_nc()
    return _NC


def _to_pt(a):
    # [L, H] -> [H(partition = l within tile), NT*H(free: tile-major, then h)]
    return np.ascontiguousarray(
        a.reshape(NT, H, H).transpose(1, 0, 2).reshape(H, L).astype(np.float32)
    )


def _masks():
    mr = np.arange(H)[:, None]
    lc0 = np.arange(512)[None, :]
    mask0 = ((lc0 >= mr - 2) & ((lc0 != mr) | (mr == 0))).astype(np.float32)
    lcc = np.arange(256)[None, :]
    maskcd = ((lcc >= mr + 126) & (lcc != mr + 128)).astype(np.float32)
    return mask0, maskcd


def make_in_maps(x, skills, individual, next_skill,
                 glu_w1, glu_b1, glu_w2, glu_b2, conv_w, conv_b):
    x = np.asarray(x, np.float32)
    skills = np.asarray(skills, np.float32)
    individual = np.asarray(individual, np.float32)
    next_skill = np.asarray(next_skill, np.float32)
    mask0, maskcd = _masks()
    w1_pt = np.ascontiguousarray(
        np.asarray(glu_w1, np.float32).reshape(3, H, H).transpose(1, 0, 2).reshape(H, 384)
    )
    w2_pt = np.ascontiguousarray(
        np.asarray(glu_w2, np.float32).reshape(3, H, H).transpose(1, 0, 2).reshape(H, 384)
    )
    cw_pt = np.ascontiguousarray(
        np.asarray(conv_w, np.float32).transpose(1, 0, 2).reshape(H, 384)
    )
    b1 = np.asarray(glu_b1, np.float32).reshape(H, 1)
    b2 = np.asarray(glu_b2, np.float32).reshape(H, 1)
    cb = np.asarray(conv_b, np.float32).reshape(H, 1)
    in_maps = []
    for b in range(B):
        in_maps.append({
            "skills_pt": _to_pt(skills[b]),
            "x_pt": _to_pt(x[b]),
            "xT": np.ascontiguousarray(x[b].T),
            "indT": np.ascontiguousarray(individual[b].T),
            "nsT": np.ascontiguousarray(next_skill[b].T),
            "mask0": mask0, "maskcd": maskcd,
            "w1_pt": w1_pt, "w2_pt": w2_pt, "cw_pt": cw_pt,
            "b1": b1, "b2": b2, "cb": cb,
        })
    return in_maps


def run(trace=False, **inputs):
    """Run on the 8 NeuronCores; returns (output [B,L,H], BassKernelResults)."""
    nc = _get_nc()
    in_maps = make_in_maps(**inputs)
    res = run_bass_kernel_spmd(nc, in_maps, list(range(B)), trace=trace)
    out = np.stack([np.ascontiguousarray(res.results[b]["outT"].T) for b in range(B)])
    return out.astype(np.float32), res


def kernel(**inputs):
    out, _ = run(trace=False, **inputs)
    return out
